# revision 10
# baseline (speedup 1.0000x reference)
"""Trainium2 Bass kernel for CustomLSTM (B=128, T=1024, D=U=256).

Strategy (data-parallel over batch, 8 cores x 16 rows):
  - Everything on-chip lives feature-major: [unit (128 partitions), batch]
    so the serial recurrence's small ops use all 128 partitions.
  - x is pre-transposed/cast to bf16 on the host ([D, T, B_local]) so the
    input projection needs no on-device transposes; W/U are bf16 too
    (fp32 PSUM accumulation), halving PE weight-load time.
  - Per T-chunk (64 steps): DMA the xT slice, run the 4-gate projection as
    W-stationary GEMMs, apply bias via ScalarE Identity-activation on the
    PSUM->SBUF copy (xproj kept fp32 in SBUF).
  - Per step: inject xproj[t] into a PSUM bank via an identity matmul
    (start=True), accumulate the 16 U-chunk matmuls (h stays feature-major
    bf16 so it feeds back as the moving operand with no transpose), then
    sigmoid/tanh on ScalarE and the elementwise c/h update on VectorE
    (c state stays fp32).
  - h history accumulates in SBUF (bf16); PE-transposes convert it back to
    batch-major fp32 for the h_all DMA, interleaved into the next chunk.
"""

import os
import numpy as np

B, T, D, NU = 128, 1024, 256, 256
NCORES = 8
BL = B // NCORES  # batch rows per core
TC = 64           # timesteps per chunk
P = 128
MC = 8            # gate-output 128-chunks (4 gates * 256 / 128)
KC = 2            # contraction 128-chunks (256 / 128)

_num_hw_cores = NCORES


def _build_nc(T_total=T, reps=1):
    import contextlib

    import ml_dtypes  # noqa: F401  (bf16 numpy dtype)
    import concourse.bass as bass
    import concourse.bacc as bacc
    import concourse.mybir as mybir
    import concourse.tile as tile

    fp32 = mybir.dt.float32
    bf16 = mybir.dt.bfloat16
    AF = mybir.ActivationFunctionType
    ts = bass.ts
    nch = T_total // TC
    assert T_total % TC == 0

    nc = bacc.Bacc("TRN2")
    xT_d = nc.dram_tensor("xT16", [D, T_total, BL], bf16, kind="ExternalInput")
    h0_d = nc.dram_tensor("h0", [BL, NU], fp32, kind="ExternalInput")
    c0_d = nc.dram_tensor("c0", [BL, NU], fp32, kind="ExternalInput")
    W_d = nc.dram_tensor("Wcat16", [D, 4 * NU], bf16, kind="ExternalInput")
    Uc_d = nc.dram_tensor("Ucat16", [NU, 4 * NU], bf16, kind="ExternalInput")
    b_d = nc.dram_tensor("bcat", [4 * NU], fp32, kind="ExternalInput")
    hall_d = nc.dram_tensor("h_all", [BL, T_total, NU], fp32, kind="ExternalOutput")
    hT_d = nc.dram_tensor("hT", [BL, NU], fp32, kind="ExternalOutput")
    cT_d = nc.dram_tensor("cT", [BL, NU], fp32, kind="ExternalOutput")
    ident_d = nc.inline_tensor(np.eye(P, dtype=np.float32), name="ident")
    ident16_d = nc.inline_tensor(
        np.eye(P, dtype=np.float32).astype(ml_dtypes.bfloat16), name="ident16"
    )

    RB = TC * BL  # columns per (k or m) block in xt/xp: (t, b) pairs

    with tile.TileContext(nc) as tc:
        with (
            tc.tile_pool(name="const", bufs=1) as constp,
            tc.tile_pool(name="xt", bufs=2) as xtp,
            tc.tile_pool(name="xp", bufs=2) as xpp,
            tc.tile_pool(name="hbuf", bufs=2) as hbufp,
            tc.tile_pool(name="hout", bufs=3) as houtp,
            tc.tile_pool(name="tail", bufs=4) as tailp,
            tc.tile_pool(name="tpps", bufs=2, space="PSUM") as tpps,
            tc.tile_pool(name="projps", bufs=2, space="PSUM") as projps,
            tc.tile_pool(name="gps", bufs=2, space="PSUM") as gps,
        ):

            def emit_body():
                ident = constp.tile([P, P], fp32)
                nc.sync.dma_start(ident[:], ident_d[:])
                ident16 = constp.tile([P, P], bf16)
                nc.sync.dma_start(ident16[:], ident16_d[:])
                U_sb = constp.tile([P, KC * 4 * NU], bf16)
                nc.sync.dma_start(U_sb[:], Uc_d.rearrange("(k p) n -> p k n", p=P))
                W_sb = constp.tile([P, KC * 4 * NU], bf16)
                nc.sync.dma_start(W_sb[:], W_d.rearrange("(k p) n -> p k n", p=P))
                bias_sb = constp.tile([P, MC], fp32)
                nc.sync.dma_start(bias_sb[:], b_d.rearrange("(m p) -> p m", p=P))

                # initial state, transposed to [unit, (k, b)]
                hinit = constp.tile([P, KC * BL], bf16)
                cstate = constp.tile([P, KC * BL], fp32)
                h0sb = constp.tile([BL, NU], fp32)
                c0sb = constp.tile([BL, NU], fp32)
                nc.sync.dma_start(h0sb[:], h0_d[:])
                nc.sync.dma_start(c0sb[:], c0_d[:])
                for k in range(KC):
                    pt = tpps.tile([P, P], fp32, tag="tp")
                    nc.tensor.transpose(pt[:, :BL], h0sb[:, ts(k, P)], ident[:BL, :BL])
                    nc.vector.tensor_copy(hinit[:, ts(k, BL)], pt[:, :BL])
                    pt2 = tpps.tile([P, P], fp32, tag="tp")
                    nc.tensor.transpose(pt2[:, :BL], c0sb[:, ts(k, P)], ident[:BL, :BL])
                    nc.vector.tensor_copy(cstate[:, ts(k, BL)], pt2[:, :BL])

                def emit_proj(ci):
                    """Allocate chunk tiles; return (xp tile, emission pieces)."""
                    t0 = ci * TC
                    xp = xpp.tile([P, MC * RB], fp32)  # col = m*RB + t*BL + b
                    xt = xtp.tile([P, KC * RB], bf16)  # col = k*RB + t*BL + b
                    pieces = []

                    def piece_load(k, xp=xp, xt=xt, t0=t0):
                        nc.sync.dma_start(
                            xt[:, ts(k, RB)], xT_d[ts(k, P), t0 : t0 + TC, :]
                        )

                    def piece_mm(m, xp=xp, xt=xt):
                        for nh in range(2):
                            pp = projps.tile([P, 512], fp32)
                            for k in range(KC):
                                nc.tensor.matmul(
                                    pp[:],
                                    W_sb[:, k * 1024 + m * P : k * 1024 + (m + 1) * P],
                                    xt[:, k * RB + nh * 512 : k * RB + (nh + 1) * 512],
                                    start=(k == 0),
                                    stop=(k == KC - 1),
                                )
                            nc.scalar.add(
                                xp[:, m * RB + nh * 512 : m * RB + (nh + 1) * 512],
                                pp[:],
                                bias_sb[:, m : m + 1],
                            )

                    for k in range(KC):
                        pieces.append(lambda k=k: piece_load(k))
                    for m in range(MC):
                        pieces.append(lambda m=m: piece_mm(m))
                    return xp, pieces

                def emit_hout_pieces(ci, hbuf):
                    t0 = ci * TC
                    pieces = []

                    def piece(j, hbuf=hbuf, t0=t0):
                        ho = houtp.tile([P, NU], fp32)
                        for uc in range(KC):
                            pt = tpps.tile([P, P], fp32, tag="tp")
                            nc.tensor.transpose(
                                pt[:],
                                hbuf[:, uc * RB + j * P : uc * RB + (j + 1) * P],
                                ident16[:],
                            )
                            nc.vector.tensor_copy(ho[:, ts(uc, P)], pt[:])
                        rpj = P // BL  # timesteps per 128-row block
                        # ho rows are (t, b) with t outer; permuted DRAM AP matches
                        nc.sync.dma_start(
                            hall_d[
                                :, t0 + j * rpj : t0 + (j + 1) * rpj, :
                            ].rearrange("b t u -> t b u"),
                            ho[:],
                        )

                    for j in range(RB // P):
                        pieces.append(lambda j=j: piece(j))
                    return pieces

                def emit_recur(ci, xp, hprev, interleave):
                    """hprev: fn k -> [128, BL] AP of h_{-1}. Returns hbuf."""
                    hbuf = hbufp.tile([P, KC * TC * BL], bf16)  # uc*RB + t*BL + b
                    xp_r = xp.rearrange("p (m t b) -> p m t b", m=MC, t=TC)
                    hb_r = hbuf.rearrange("p (uc t b) -> p uc t b", uc=KC, t=TC)
                    for t in range(TC):
                        if t % 2 == 0 and interleave:
                            interleave.pop(0)()
                        if t == 0:
                            hsrc = hprev
                        else:
                            def hsrc(k, t=t):
                                return hbuf[
                                    :, k * RB + (t - 1) * BL : k * RB + t * BL
                                ]
                        G = gps.tile([P, MC * BL], fp32)
                        # xproj[t] -> PSUM via identity matmul (sets has_written)
                        nc.tensor.matmul(
                            G[:], ident[:], xp_r[:, :, t, :], start=True, stop=False
                        )
                        for m in range(MC):
                            for k in range(KC):
                                nc.tensor.matmul(
                                    G[:, ts(m, BL)],
                                    U_sb[:, k * 1024 + m * P : k * 1024 + (m + 1) * P],
                                    hsrc(k),
                                    start=False,
                                    stop=(m == MC - 1 and k == KC - 1),
                                )
                        S = tailp.tile([P, 3 * KC * BL], fp32, tag="S")
                        nc.scalar.activation(S[:], G[:, 0 : 3 * KC * BL], AF.Sigmoid)
                        Ch = tailp.tile([P, KC * BL], fp32, tag="Ch")
                        nc.scalar.activation(
                            Ch[:], G[:, 3 * KC * BL : 4 * KC * BL], AF.Tanh
                        )
                        t1 = tailp.tile([P, KC * BL], fp32, tag="t1")
                        nc.vector.tensor_mul(t1[:], S[:, 0 : KC * BL], cstate[:])
                        t2 = tailp.tile([P, KC * BL], fp32, tag="t2")
                        nc.vector.tensor_mul(t2[:], S[:, KC * BL : 2 * KC * BL], Ch[:])
                        nc.vector.tensor_add(cstate[:], t1[:], t2[:])
                        Th = tailp.tile([P, KC * BL], fp32, tag="Th")
                        nc.scalar.activation(Th[:], cstate[:], AF.Tanh)
                        o_r = S.rearrange("p (g uc b) -> p g uc b", g=3, uc=KC)
                        th_r = Th.rearrange("p (uc b) -> p uc b", uc=KC)
                        nc.vector.tensor_mul(hb_r[:, :, t, :], o_r[:, 2], th_r[:])
                    while interleave:
                        interleave.pop(0)()
                    return hbuf

                xp_cur, p0 = emit_proj(0)
                for pc in p0:
                    pc()
                hbufs = []
                hprev = lambda k: hinit[:, ts(k, BL)]
                for ci in range(nch):
                    inter = []
                    if ci > 0:
                        inter += emit_hout_pieces(ci - 1, hbufs[ci - 1])
                    xp_next = None
                    if ci + 1 < nch:
                        xp_next, pn = emit_proj(ci + 1)
                        inter += pn
                    hb = emit_recur(ci, xp_cur, hprev, inter)
                    hbufs.append(hb)
                    xp_cur = xp_next
                    hprev = lambda k, hb=hb: hb[
                        :, k * RB + (TC - 1) * BL : k * RB + TC * BL
                    ]
                for pc in emit_hout_pieces(nch - 1, hbufs[-1]):
                    pc()

                hT_sb = constp.tile([BL, NU], fp32)
                cT_sb = constp.tile([BL, NU], fp32)
                for k in range(KC):
                    pt = tpps.tile([P, P], fp32, tag="tp")
                    nc.tensor.transpose(
                        pt[:BL, :],
                        hbufs[-1][:, k * RB + (TC - 1) * BL : k * RB + TC * BL],
                        ident16[:],
                    )
                    nc.vector.tensor_copy(hT_sb[:, ts(k, P)], pt[:BL, :])
                    pt2 = tpps.tile([P, P], fp32, tag="tp")
                    nc.tensor.transpose(pt2[:BL, :], cstate[:, ts(k, BL)], ident[:])
                    nc.vector.tensor_copy(cT_sb[:, ts(k, P)], pt2[:BL, :])
                nc.sync.dma_start(hT_d[:], hT_sb[:])
                nc.sync.dma_start(cT_d[:], cT_sb[:])

            # reps>1 repeats the whole computation on-device (for timing runs)
            rep_ctx = tc.For_i(0, reps, 1) if reps > 1 else contextlib.nullcontext()
            with rep_ctx:
                emit_body()

    nc.finalize()
    return nc


def _make_in_maps(inputs):
    import ml_dtypes

    f32 = np.float32
    bf = ml_dtypes.bfloat16
    x = np.asarray(inputs["x"], dtype=f32)
    h0 = np.ascontiguousarray(np.asarray(inputs["h0"], dtype=f32))
    c0 = np.ascontiguousarray(np.asarray(inputs["c0"], dtype=f32))
    xT16 = np.ascontiguousarray(x.transpose(2, 1, 0).astype(bf))  # [D, T, B]
    # gate order: f, i, o, c (sigmoid gates first, tanh candidate last)
    Wcat16 = np.ascontiguousarray(
        np.concatenate(
            [inputs["Wf"], inputs["Wi"], inputs["Wo"], inputs["Wc"]], axis=1
        ).astype(bf)
    )
    Ucat16 = np.ascontiguousarray(
        np.concatenate(
            [inputs["Uf"], inputs["Ui"], inputs["Uo"], inputs["Uc"]], axis=1
        ).astype(bf)
    )
    bcat = np.ascontiguousarray(
        np.concatenate(
            [inputs["bf"], inputs["bi"], inputs["bo"], inputs["bc"]]
        ).astype(f32)
    )
    in_maps = []
    for c in range(_num_hw_cores):
        sl = slice(c * BL, (c + 1) * BL)
        in_maps.append(
            {
                "xT16": np.ascontiguousarray(xT16[:, :, sl]),
                "h0": np.ascontiguousarray(h0[sl]),
                "c0": np.ascontiguousarray(c0[sl]),
                "Wcat16": Wcat16,
                "Ucat16": Ucat16,
                "bcat": bcat,
            }
        )
    return in_maps


def _run(inputs, T_total=T, trace=False, reps=1):
    # The kernel executes through jax/PJRT on the neuron (axon) platform;
    # a leftover JAX_PLATFORMS=cpu (used for running the reference) breaks it.
    import sys
    if "jax" not in sys.modules and os.environ.get("JAX_PLATFORMS") == "cpu":
        os.environ.pop("JAX_PLATFORMS")
    from concourse import bass_utils

    nc = _build_nc(T_total, reps=reps)
    in_maps = _make_in_maps(inputs)
    res = bass_utils.run_bass_kernel_spmd(
        nc, in_maps, core_ids=list(range(_num_hw_cores)), trace=trace
    )
    h_all = np.concatenate([r["h_all"] for r in res.results], axis=0)
    hT = np.concatenate([r["hT"] for r in res.results], axis=0)
    cT = np.concatenate([r["cT"] for r in res.results], axis=0)
    return (h_all, hT, cT), res


def kernel(**inputs):
    (h_all, hT, cT), _ = _run(inputs, T_total=T, trace=False)
    return h_all, hT, cT
